# revision 15
# baseline (speedup 1.0000x reference)
"""Multi-head causal attention on 8 Trainium2 NeuronCores.

Sharding: core c -> batch b = c // 4, head-group g = c % 4 (4 of 16 heads).
Each core computes its 4 heads' attention and the partial W_O contraction;
the host sums the 4 head-group partials per batch (the reduce of the
tensor-parallel split).

Device-side layout is transpose-free: the host pre-transposes x and the
weights so every matmul contraction lands on the partition axis:
  qT[e,s], kT[e,s]  = W^T-chunk.T @ xT-chunk          (accum over d)
  v[m,he]           = xT-chunk.T @ WvT-chunk          (accum over d)
  sT[m,s]           = kT-slice.T @ qT-block           (scores, transposed)
  pT[m,s]           = exp(sT * 1/sqrt(e))  * mask     (ScalarE + DVE)
  zT[e,s]          += v-slice.T @ pT                  (accum over m)
  den[1,s]         += ones.T @ pT                     (softmax denominator)
  recip             = 1/den                           (DVE)
  zn[e,s]           = zT * (ones x recip)             (PE outer-prod bcast)
  out[s,d]         += zn-slice.T @ WoT                (accum over heads)

Schedule: the PE is the bottleneck engine (~290us of fp16 matmul at
2.4GHz), so everything else is interleaved INTO the PE instruction
stream to keep it saturated:
  - attention block j=0 is interleaved into projection blocks j'=1..3
    (projection matmuls hide the ScalarE exp latency),
  - the output projection for block j-1 is interleaved into the
    attention chunk stream of block j (out-mms fill the gap between
    PE chunk work ~530ns and ScalarE exp ~690ns per chunk),
  - only out(3) runs as a clean tail at full PE issue rate.

All matmul operands fp16 (full PE rate), accumulation fp32 in PSUM.
"""

import math

import numpy as np

B = 2
S = 2048
D = 2048
H = 16
E = 128
HPC = 4          # heads per core
HE = HPC * E     # 512
NC_CHUNKS = D // 128   # 16 contraction chunks of 128
NBLK = 4         # s-blocks of 512
NMT = S // 128   # 16 m-tiles of 128
SCALE = 1.0 / math.sqrt(E)
N_CORES = 8

_CACHE = {}


def _build_program():
    import concourse.bacc as bacc
    import concourse.mybir as mybir
    import concourse.tile as tile

    f16 = mybir.dt.float16
    f32 = mybir.dt.float32
    Exp = mybir.ActivationFunctionType.Exp

    nc = bacc.Bacc("TRN2", target_bir_lowering=False, debug=False,
                   num_devices=N_CORES)

    xT_d = nc.dram_tensor("xT", [D, S], f16, kind="ExternalInput")
    wq_d = nc.dram_tensor("wq", [D, HE], f16, kind="ExternalInput")
    wk_d = nc.dram_tensor("wk", [D, HE], f16, kind="ExternalInput")
    wv_d = nc.dram_tensor("wv", [D, HE], f16, kind="ExternalInput")
    woT_d = nc.dram_tensor("woT", [HE, D], f16, kind="ExternalInput")
    masks_d = nc.dram_tensor("masks", [128, 128], f16, kind="ExternalInput")
    outp_d = nc.dram_tensor("outp", [S, D], f16, kind="ExternalOutput")

    with tile.TileContext(nc) as tc:
        with (
            tc.tile_pool(name="const", bufs=1) as constp,
            tc.tile_pool(name="qkv", bufs=1) as qkvp,
            tc.tile_pool(name="post", bufs=2) as postp,
            tc.tile_pool(name="work", bufs=2) as workp,
            tc.tile_pool(name="osb", bufs=4) as osbp,
            tc.tile_pool(name="pt", bufs=8) as ptp,
        ):
            # ones via memset: no DMA dependency, so PE warm-up matmuls
            # can start right after the framework preamble (~6us) instead
            # of waiting for the first DMA completion (~9.4us)
            ones_sb = constp.tile([128, 129], f16, tag="ones")
            nc.gpsimd.memset(ones_sb[:], 1.0)
            onesm = ones_sb[:, 0:1]            # [128, 1] denominator lhsT
            onescol = ones_sb[0:1, 1:129]      # [1, 128] broadcast lhsT
            woT_sb = constp.tile([128, HPC, D], f16, tag="woT")
            masks_sb = constp.tile([128, 128], f16, tag="masks")

            qT = [qkvp.tile([128, S], f16, tag=f"qT{h}", name=f"qT{h}")
                  for h in range(HPC)]
            kT = [qkvp.tile([128, S], f16, tag=f"kT{h}", name=f"kT{h}")
                  for h in range(HPC)]
            vt = [qkvp.tile([128, HE], f16, tag=f"v{m}", name=f"v{m}")
                  for m in range(NMT)]

            zn = [[None] * NBLK for _ in range(HPC)]

            # ---------------- attention block generator ----------------
            # Emits the full attention for (j, h); yields after each PE
            # quantum so the caller can interleave independent matmuls.
            # The end-of-head normalize chain is split: the DVE reciprocal
            # is emitted inline, but the PE broadcast matmul + zn multiply
            # are pushed into `defer` (fired by the caller a few PE quanta
            # later) so the PE never sits waiting on the DVE chain.
            def attn_head(j, h, psS, psZ, psM, defer):
                zps = psZ.tile([128, 512], f32, tag="z")
                dps = psM.tile([1, 512], f32, tag="m")
                nchunks = 4 * j + 4
                # software pipeline: scores/exp run `off` chunks ahead
                # of PV/den so the PE never waits on a fresh exp
                pts = [None] * nchunks
                cols = [None] * nchunks

                def emit_score(i):
                    # columns < c0 are fully masked (never read)
                    r = i - 4 * j
                    c0 = 128 * r if r > 0 else 0
                    cols[i] = c0
                    sps = psS.tile([128, 512], f32, tag="s", name="sps")
                    nc.tensor.matmul(
                        sps[:, c0:512],
                        lhsT=kT[h][:, i * 128:(i + 1) * 128],
                        rhs=qT[h][:, j * 512 + c0:(j + 1) * 512],
                        start=True, stop=True)
                    pt = ptp.tile([128, 512], f16, tag="pt", name="pt")
                    nc.scalar.activation(pt[:, c0:512],
                                         sps[:, c0:512], Exp,
                                         scale=SCALE)
                    if r >= 0:
                        # only the 128-wide diagonal band is partially
                        # masked; the multiply runs on the mostly-idle
                        # GpSimd so the PV matmul never waits behind the
                        # busy DVE queue for it
                        nc.gpsimd.tensor_mul(
                            pt[:, c0:c0 + 128], pt[:, c0:c0 + 128],
                            masks_sb[:])
                    if r in (1, 3):
                        # zero the dead band so this chunk can be
                        # pair-summed with its even partner for
                        # the denominator
                        nc.gpsimd.memset(pt[:, c0 - 128:c0], 0.0)
                    pts[i] = pt

                den_state = {"started": False, "held": None,
                             "heldsum": None}

                def emit_pv(i):
                    c0 = cols[i]
                    pt = pts[i]
                    last = (i == nchunks - 1)
                    nc.tensor.matmul(
                        zps[:, c0:512],
                        lhsT=vt[i][:, h * E:(h + 1) * E],
                        rhs=pt[:, c0:512], start=(i == 0), stop=last,
                        skip_group_check=(c0 > 0))
                    # denominator: full chunks are summed in pairs
                    # on the (half-idle) DVE so one ones-matmul
                    # covers two chunks; diagonal chunks go solo
                    if i < 4 * j:
                        if den_state["held"] is None:
                            den_state["held"] = pt
                        else:
                            ptsum = workp.tile([128, 512], f16,
                                               tag="ptsum",
                                               name="ptsum",
                                               bufs=3)
                            nc.vector.tensor_add(
                                ptsum[:], den_state["held"][:],
                                pt[:])
                            den_state["held"] = None
                            if den_state["heldsum"] is None:
                                den_state["heldsum"] = ptsum
                            else:
                                qsum = workp.tile([128, 512], f16,
                                                  tag="qsum",
                                                  name="qsum")
                                nc.vector.tensor_add(
                                    qsum[:],
                                    den_state["heldsum"][:],
                                    ptsum[:])
                                den_state["heldsum"] = None
                                nc.tensor.matmul(
                                    dps[:], lhsT=onesm,
                                    rhs=qsum[:],
                                    start=not den_state["started"],
                                    stop=False)
                                den_state["started"] = True
                    elif (i - 4 * j) in (0, 2):
                        den_state["held"] = pt
                    else:
                        base = cols[i - 1]
                        dsum = workp.tile([128, 512], f16,
                                          tag="dsum", name="dsum")
                        nc.vector.tensor_add(
                            dsum[:, base:512],
                            den_state["held"][:, base:512],
                            pt[:, base:512])
                        den_state["held"] = None
                        nc.tensor.matmul(
                            dps[:, base:512], lhsT=onesm,
                            rhs=dsum[:, base:512],
                            start=not den_state["started"],
                            stop=last,
                            skip_group_check=(base > 0))
                        den_state["started"] = True
                    pts[i] = None

                off = min(4, nchunks - 1)
                for i in range(nchunks):
                    emit_score(i)
                    if i >= off:
                        emit_pv(i - off)
                    yield
                for i in range(nchunks - off, nchunks):
                    emit_pv(i)
                    yield
                rec32 = workp.tile([1, 512], f32, tag="rec32")
                nc.vector.reciprocal_approx_fast(rec32[:], dps[:])
                rec = workp.tile([1, 512], f16, tag="rec")
                nc.vector.tensor_copy(rec[:], rec32[:])

                def finish(h=h, j=j, zps=zps, rec=rec):
                    # broadcast of the reciprocal row across partitions on
                    # the (otherwise idle) GpSimd engine: frees the PE
                    # matmul + PSUM bank + DVE cast the old PE-broadcast
                    # needed
                    bsb = workp.tile([128, 512], f16, tag="bsb")
                    nc.gpsimd.partition_broadcast(bsb[:], rec[:])
                    z = postp.tile([128, 512], f16, tag=f"zn{h}",
                                   name=f"zn{h}_{j}")
                    nc.vector.tensor_mul(z[:], zps[:], bsb[:])
                    zn[h][j] = z

                defer.append([3, finish])
                yield

            # ------------- output-projection block generator -------------
            # 64 matmuls (16 groups of 4-head accumulation); yields after
            # each matmul. Copy+DMA per group emitted inline. The final
            # group of the final block splits its copy+DMA in two halves
            # issued on separate queues to shorten the exposed tail.
            def out_block(j, pool, fine_tail=False,
                          scalar_copies_from=99):
                Copy = mybir.ActivationFunctionType.Copy
                gidx = [0]

                def copy_out(osb_ap, src_ap):
                    # tail-region copies run on the (idle) ScalarE so the
                    # DVE FIFO holds only the recip->mul chain of the
                    # final head and never head-of-line blocks the PSUM
                    # bank recycling of these groups
                    if gidx[0] >= scalar_copies_from or fine_tail:
                        nc.scalar.activation(osb_ap, src_ap, Copy)
                    else:
                        nc.vector.tensor_copy(osb_ap, src_ap)

                def mm(ops, st, db, h):
                    nc.tensor.matmul(
                        ops[:],
                        lhsT=zn[h][j][:, st * 128:(st + 1) * 128],
                        rhs=woT_sb[:, h, db * 512:(db + 1) * 512],
                        start=(h == 0), stop=(h == HPC - 1))

                held = {}
                if fine_tail:
                    # fill both PSUM banks with the h0-2 partials of the
                    # first two groups while the last head's zn normalize
                    # chain (recip->broadcast->mul) completes
                    for db in range(2):
                        held[db] = pool.tile([128, 512], f32, tag="o",
                                             name="ops")
                        for h in range(HPC - 1):
                            mm(held[db], 0, db, h)
                            yield
                for st in range(4):
                    for db in range(4):
                        if (st, db) in ((0, 0), (0, 1)) and fine_tail:
                            ops = held[db]
                            mm(ops, st, db, HPC - 1)
                            yield
                        else:
                            ops = pool.tile([128, 512], f32, tag="o",
                                            name="ops")
                            for h in range(HPC):
                                mm(ops, st, db, h)
                                yield
                        row = j * 512 + st * 128
                        last = fine_tail and st == 3 and db == 3
                        gidx[0] += 1
                        if last:
                            for piece, eng in ((0, nc.sync),
                                               (1, nc.scalar)):
                                osb = osbp.tile([128, 256], f16,
                                                tag="osbf", name="osbf")
                                lo = piece * 256
                                copy_out(osb[:], ops[:, lo:lo + 256])
                                eng.dma_start(
                                    outp_d[row:row + 128,
                                           db * 512 + lo:
                                           db * 512 + lo + 256], osb[:])
                        else:
                            osb = osbp.tile([128, 512], f16, tag="osb",
                                            name="osb")
                            copy_out(osb[:], ops[:])
                            nc.sync.dma_start(
                                outp_d[row:row + 128,
                                       db * 512:(db + 1) * 512], osb[:])

            with (
                tc.tile_pool(name="psS", bufs=3, space="PSUM") as psS,
                tc.tile_pool(name="psZ", bufs=2, space="PSUM") as psZ,
                tc.tile_pool(name="psM", bufs=1, space="PSUM") as psM,
            ):
                # ---- Phase B: projections, with attn(0) interleaved
                with (
                    tc.tile_pool(name="big", bufs=1) as bigp,
                    tc.tile_pool(name="psumB", bufs=2,
                                 space="PSUM") as psB,
                ):
                    # xT is a per-j-block rotating buffer (bufs=2): block
                    # j's slice is only read by projection block j, and
                    # the j>=2 loads go on the SYNC queue only — they wait
                    # for proj(j-2) consumers, and putting them on the
                    # scalar queue would stall attn(0)'s exp stream
                    # behind them (deadlock against PE program order).
                    src = xT_d.rearrange("(c p) s -> p c s", p=128)
                    xTt = [bigp.tile([128, NC_CHUNKS, 512], f16, tag="xT",
                                     bufs=2, name=f"xT{j}")
                           for j in range(NBLK)]
                    w_sb = {}
                    for name, dram in (("wq", wq_d), ("wk", wk_d),
                                       ("wv", wv_d)):
                        w_sb[name] = bigp.tile([128, NC_CHUNKS, HE], f16,
                                               tag=name, name=name + "_sb")
                    wsrc = {name: dram.rearrange("(c p) n -> p c n", p=128)
                            for name, dram in
                            (("wq", wq_d), ("wk", wk_d), ("wv", wv_d))}

                    def load_w(eng, name, piece, npieces=2):
                        w = NC_CHUNKS // npieces
                        eng.dma_start(
                            w_sb[name][:, w * piece:w * piece + w, :],
                            wsrc[name][:, w * piece:w * piece + w, :])

                    def load_x(eng, j, half):
                        eng.dma_start(
                            xTt[j][:, 8 * half:8 * half + 8, :],
                            src[:, 8 * half:8 * half + 8,
                                j * 512:(j + 1) * 512])

                    # dummy matmuls on the ones tile while input DMAs run:
                    # keeps the PE-HAM activity window busy so the real
                    # matmul stream starts at the full 2.4GHz clock
                    for w in range(24):
                        wps = psB.tile([1, 128], f32, tag="proj",
                                       name="wps")
                        nc.tensor.matmul(wps[:], lhsT=onesm,
                                         rhs=ones_sb[:, 1:129],
                                         start=True, stop=True)

                    # first-needed data in small pieces, issued from BOTH
                    # hwdge queues (sync + scalar) so neither the ~700ns
                    # per-issue cost nor one queue's serial drain gates
                    # the first matmul chain
                    for c in range(4):
                        nc.sync.dma_start(w_sb["wk"][:, c:c + 1, :],
                                          wsrc["wk"][:, c:c + 1, :])
                        nc.scalar.dma_start(xTt[0][:, c:c + 1, :],
                                            src[:, c:c + 1, 0:512])
                    # second warm-up batch, gated on the first xT chunk so
                    # PE activity bridges the gap until projections begin
                    for w in range(16):
                        wps = psB.tile([1, 128], f32, tag="proj",
                                       name="wps")
                        nc.tensor.matmul(wps[:], lhsT=onesm,
                                         rhs=xTt[0][:, 0, 0:128],
                                         start=True, stop=True)
                    for e6 in range(2, 8):
                        nc.sync.dma_start(
                            w_sb["wk"][:, 2 * e6:2 * e6 + 2, :],
                            wsrc["wk"][:, 2 * e6:2 * e6 + 2, :])
                        nc.scalar.dma_start(
                            xTt[0][:, 2 * e6:2 * e6 + 2, :],
                            src[:, 2 * e6:2 * e6 + 2, 0:512])
                    for q in range(4):
                        load_w(nc.sync, "wq", q, 4)
                    nc.scalar.dma_start(masks_sb[:], masks_d[:])
                    for q in range(4):
                        load_w(nc.scalar, "wv", q, 4)
                    load_x(nc.sync, 1, 0)
                    load_x(nc.scalar, 1, 1)
                    wot_src = woT_d.rearrange("(c p) d -> p c d", p=128)
                    for c in range(HPC):
                        nc.sync.dma_start(woT_sb[:, c, :],
                                          wot_src[:, c, :])
                    for j in range(2, NBLK):
                        load_x(nc.sync, j, 0)
                        load_x(nc.sync, j, 1)

                    def proj(dst_ap, lhs_of_c, rhs_of_c):
                        ps = psB.tile([128, 512], f32, tag="proj",
                                      name="ps")
                        for c in range(NC_CHUNKS):
                            nc.tensor.matmul(
                                ps[:], lhsT=lhs_of_c(c), rhs=rhs_of_c(c),
                                start=(c == 0), stop=(c == NC_CHUNKS - 1))
                        nc.vector.tensor_copy(dst_ap, ps[:])

                    # attention block 0 interleaves into proj blocks 1..3:
                    # one attention quantum between projection chains
                    defer0 = []
                    gen0s = [attn_head(0, h, psS, psZ, psM, defer0)
                             for h in range(HPC)]
                    gen0i = 0

                    def step0():
                        nonlocal gen0i
                        while gen0i < len(gen0s):
                            if next(gen0s[gen0i], StopIteration) \
                                    is not StopIteration:
                                tick(defer0)
                                return
                            gen0i += 1

                    for j in range(NBLK):
                        if j == 0:
                            kq_order = [(kT[h], w_sb["wk"], h)
                                        for h in range(HPC)]
                            kq_order += [(qT[h], w_sb["wq"], h)
                                         for h in range(HPC)]
                        else:
                            kq_order = [p for h in range(HPC)
                                        for p in ((kT[h], w_sb["wk"], h),
                                                  (qT[h], w_sb["wq"], h))]
                        for dst, w, h in kq_order:
                            proj(dst[:, j * 512:(j + 1) * 512],
                                 lambda c, w=w, h=h:
                                 w[:, c, h * E:(h + 1) * E],
                                 lambda c, j=j: xTt[j][:, c, :])
                            if j >= 1:
                                step0()
                        for m in range(4 * j, 4 * j + 4):
                            proj(vt[m][:],
                                 lambda c, m=m, j=j:
                                 xTt[j][:, c,
                                        (m - 4 * j) * 128:
                                        (m - 4 * j + 1) * 128],
                                 lambda c: w_sb["wv"][:, c, :])
                            if j >= 1:
                                step0()
                    # drain whatever is left of attn(0)
                    while gen0i < len(gen0s):
                        step0()
                    flush(defer0)

                # ---- attention blocks 1..3 with out(j-1) interleaved,
                # then out(3) as a clean PE-bound tail
                with tc.tile_pool(name="psO", bufs=2,
                                  space="PSUM") as psO:
                    for j in range(1, NBLK):
                        og = out_block(j - 1, psO,
                                       scalar_copies_from=13
                                       if j == NBLK - 1 else 99)
                        odone = 0
                        # 72/64 pacing: out(j-1) finishes ~88% through the
                        # stage so its tail never collides with the next
                        # stage's start
                        nyield = 4 * (4 * j + 4 + off_yields(j))
                        ydone = 0
                        defer = []
                        for h in range(HPC):
                            for _ in attn_head(j, h, psS, psZ, psM,
                                               defer):
                                ydone += 1
                                tick(defer)
                                # stage 3 holds back ~8 out(2) matmuls:
                                # they are drained right after the last
                                # attention chunk, filling the PE while
                                # the final head's recip->broadcast->mul
                                # chain (~3us serial) completes
                                cap = 52 if j == NBLK - 1 else 64
                                target = min(cap, (72 * ydone) // nyield)
                                while odone < target:
                                    if next(og, StopIteration) \
                                            is StopIteration:
                                        odone = 64
                                    else:
                                        odone += 1
                        for _ in og:
                            pass
                        flush(defer)
                    for _ in out_block(NBLK - 1, psO, fine_tail=True):
                        pass

    nc.compile()
    return nc


def tick(defer):
    """Advance deferred emissions by one PE quantum; fire due ones."""
    for it in defer:
        it[0] -= 1
    while defer and defer[0][0] <= 0:
        defer.pop(0)[1]()


def flush(defer):
    while defer:
        defer.pop(0)[1]()


def off_yields(j):
    """Extra yields of attn_head(j, h) beyond one per chunk: the
    pipeline-drain PVs (off) plus the end-of-head chain yield."""
    return min(4, 4 * j + 3) + 1


def _get_nc():
    if "nc" not in _CACHE:
        _CACHE["nc"] = _build_program()
    return _CACHE["nc"]


def _host_inputs(x, W_Q, W_K, W_V, W_O):
    """Per-core input dicts (all fp16, pre-transposed)."""
    cc = np.arange(128)[None, :]
    mm = np.arange(128)[:, None]
    masks = (cc >= mm).astype(np.float16)

    in_maps = []
    for c in range(N_CORES):
        b, g = divmod(c, 4)
        hs = slice(HPC * g, HPC * g + HPC)
        xT = np.ascontiguousarray(x[b].T).astype(np.float16)
        wq = np.ascontiguousarray(
            W_Q[hs].transpose(2, 0, 1).reshape(D, HE)).astype(np.float16)
        wk = np.ascontiguousarray(
            W_K[hs].transpose(2, 0, 1).reshape(D, HE)).astype(np.float16)
        wv = np.ascontiguousarray(
            W_V[hs].transpose(2, 0, 1).reshape(D, HE)).astype(np.float16)
        woT = np.ascontiguousarray(
            W_O[hs].transpose(0, 2, 1).reshape(HE, D)).astype(np.float16)
        in_maps.append({"xT": xT, "wq": wq, "wk": wk, "wv": wv,
                        "woT": woT, "masks": masks})
    return in_maps


def _run(in_maps, trace=False, **kw):
    from concourse.bass_utils import run_bass_kernel_spmd
    nc = _get_nc()
    return run_bass_kernel_spmd(nc, in_maps, list(range(N_CORES)),
                                trace=trace, **kw)


def kernel(x, W_Q, W_K, W_V, W_O):
    x, W_Q, W_K, W_V, W_O = (np.asarray(a, dtype=np.float32)
                             for a in (x, W_Q, W_K, W_V, W_O))
    res = _run(_host_inputs(x, W_Q, W_K, W_V, W_O))
    parts = [np.asarray(res.results[c]["outp"], dtype=np.float32)
             for c in range(N_CORES)]
    out = np.stack([parts[0] + parts[1] + parts[2] + parts[3],
                    parts[4] + parts[5] + parts[6] + parts[7]])
    return out


# revision 16
# speedup vs baseline: 1.3603x; 1.3603x over previous
"""Multi-head causal attention on 8 Trainium2 NeuronCores.

Sharding: core c -> batch b = c // 4, head-group g = c % 4 (4 of 16 heads).
Each core computes its 4 heads' attention and the partial W_O contraction;
the host sums the 4 head-group partials per batch (the reduce of the
tensor-parallel split).

Device-side layout is transpose-free: the host pre-transposes x and the
weights so every matmul contraction lands on the partition axis:
  qT[e,s], kT[e,s]  = W^T-chunk.T @ xT-chunk          (accum over d)
  v[m,he]           = xT-chunk.T @ WvT-chunk          (accum over d)
  sT[m,s]           = kT-slice.T @ qT-block           (scores, transposed)
  pT[m,s]           = exp(sT * 1/sqrt(e))  * mask     (ScalarE + DVE)
  zT[e,s]          += v-slice.T @ pT                  (accum over m)
  den[1,s]         += ones.T @ pT                     (softmax denominator)
  recip             = 1/den                           (DVE)
  zn[e,s]           = zT * (ones x recip)             (PE outer-prod bcast)
  out[s,d]         += zn-slice.T @ WoT                (accum over heads)

Schedule: the PE is the bottleneck engine (~290us of fp16 matmul at
2.4GHz), so everything else is interleaved INTO the PE instruction
stream to keep it saturated:
  - attention block j=0 is interleaved into projection blocks j'=1..3
    (projection matmuls hide the ScalarE exp latency),
  - the output projection for block j-1 is interleaved into the
    attention chunk stream of block j (out-mms fill the gap between
    PE chunk work ~530ns and ScalarE exp ~690ns per chunk),
  - only out(3) runs as a clean tail at full PE issue rate.

All matmul operands fp16 (full PE rate), accumulation fp32 in PSUM.
"""

import math

import numpy as np

B = 2
S = 2048
D = 2048
H = 16
E = 128
HPC = 4          # heads per core
HE = HPC * E     # 512
NC_CHUNKS = D // 128   # 16 contraction chunks of 128
NBLK = 4         # s-blocks of 512
NMT = S // 128   # 16 m-tiles of 128
SCALE = 1.0 / math.sqrt(E)
N_CORES = 8

_CACHE = {}


def _build_program():
    import concourse.bacc as bacc
    import concourse.mybir as mybir
    import concourse.tile as tile

    f16 = mybir.dt.float16
    f32 = mybir.dt.float32
    Exp = mybir.ActivationFunctionType.Exp

    nc = bacc.Bacc("TRN2", target_bir_lowering=False, debug=False,
                   num_devices=N_CORES)

    xT_d = nc.dram_tensor("xT", [D, S], f16, kind="ExternalInput")
    wq_d = nc.dram_tensor("wq", [D, HE], f16, kind="ExternalInput")
    wk_d = nc.dram_tensor("wk", [D, HE], f16, kind="ExternalInput")
    wv_d = nc.dram_tensor("wv", [D, HE], f16, kind="ExternalInput")
    woT_d = nc.dram_tensor("woT", [HE, D], f16, kind="ExternalInput")
    masks_d = nc.dram_tensor("masks", [128, 128], f16, kind="ExternalInput")
    outp_d = nc.dram_tensor("outp", [S, D], f16, kind="ExternalOutput")

    with tile.TileContext(nc) as tc:
        with (
            tc.tile_pool(name="const", bufs=1) as constp,
            tc.tile_pool(name="qkv", bufs=1) as qkvp,
            tc.tile_pool(name="post", bufs=2) as postp,
            tc.tile_pool(name="work", bufs=2) as workp,
            tc.tile_pool(name="osb", bufs=4) as osbp,
            tc.tile_pool(name="pt", bufs=8) as ptp,
        ):
            # ones via memset: no DMA dependency, so PE warm-up matmuls
            # can start right after the framework preamble (~6us) instead
            # of waiting for the first DMA completion (~9.4us)
            ones_sb = constp.tile([128, 129], f16, tag="ones")
            nc.gpsimd.memset(ones_sb[:], 1.0)
            onesm = ones_sb[:, 0:1]            # [128, 1] denominator lhsT
            onescol = ones_sb[0:1, 1:129]      # [1, 128] broadcast lhsT
            woT_sb = constp.tile([128, HPC, D], f16, tag="woT")
            masks_sb = constp.tile([128, 128], f16, tag="masks")

            qT = [qkvp.tile([128, S], f16, tag=f"qT{h}", name=f"qT{h}")
                  for h in range(HPC)]
            kT = [qkvp.tile([128, S], f16, tag=f"kT{h}", name=f"kT{h}")
                  for h in range(HPC)]
            vt = [qkvp.tile([128, HE], f16, tag=f"v{m}", name=f"v{m}")
                  for m in range(NMT)]

            zn = [[None] * NBLK for _ in range(HPC)]

            # ---------------- attention block generator ----------------
            # Emits the full attention for (j, h); yields after each PE
            # quantum so the caller can interleave independent matmuls.
            # The end-of-head normalize chain is split: the DVE reciprocal
            # is emitted inline, but the PE broadcast matmul + zn multiply
            # are pushed into `defer` (fired by the caller a few PE quanta
            # later) so the PE never sits waiting on the DVE chain.
            def attn_head(j, h, psS, psZ, psM, defer):
                zps = psZ.tile([128, 512], f32, tag="z")
                dps = psM.tile([1, 512], f32, tag="m")
                nchunks = 4 * j + 4
                # software pipeline: scores/exp run `off` chunks ahead
                # of PV/den so the PE never waits on a fresh exp
                pts = [None] * nchunks
                cols = [None] * nchunks

                def emit_score(i):
                    # columns < c0 are fully masked (never read)
                    r = i - 4 * j
                    c0 = 128 * r if r > 0 else 0
                    cols[i] = c0
                    sps = psS.tile([128, 512], f32, tag="s", name="sps")
                    nc.tensor.matmul(
                        sps[:, c0:512],
                        lhsT=kT[h][:, i * 128:(i + 1) * 128],
                        rhs=qT[h][:, j * 512 + c0:(j + 1) * 512],
                        start=True, stop=True)
                    pt = ptp.tile([128, 512], f16, tag="pt", name="pt")
                    nc.scalar.activation(pt[:, c0:512],
                                         sps[:, c0:512], Exp,
                                         scale=SCALE)
                    if r >= 0:
                        # only the 128-wide diagonal band is
                        # partially masked (NOTE: must stay on DVE —
                        # gpsimd tensor ops force a ucode library swap
                        # against partition_broadcast, ~5us per switch)
                        nc.vector.tensor_mul(
                            pt[:, c0:c0 + 128], pt[:, c0:c0 + 128],
                            masks_sb[:])
                    if r in (1, 3):
                        # zero the dead band so this chunk can be
                        # pair-summed with its even partner for
                        # the denominator
                        nc.gpsimd.memset(pt[:, c0 - 128:c0], 0.0)
                    pts[i] = pt

                den_state = {"started": False, "held": None,
                             "heldsum": None}

                def emit_pv(i):
                    c0 = cols[i]
                    pt = pts[i]
                    last = (i == nchunks - 1)
                    nc.tensor.matmul(
                        zps[:, c0:512],
                        lhsT=vt[i][:, h * E:(h + 1) * E],
                        rhs=pt[:, c0:512], start=(i == 0), stop=last,
                        skip_group_check=(c0 > 0))
                    # denominator: full chunks are summed in pairs
                    # on the (half-idle) DVE so one ones-matmul
                    # covers two chunks; diagonal chunks go solo
                    if i < 4 * j:
                        if den_state["held"] is None:
                            den_state["held"] = pt
                        else:
                            ptsum = workp.tile([128, 512], f16,
                                               tag="ptsum",
                                               name="ptsum",
                                               bufs=3)
                            nc.vector.tensor_add(
                                ptsum[:], den_state["held"][:],
                                pt[:])
                            den_state["held"] = None
                            if den_state["heldsum"] is None:
                                den_state["heldsum"] = ptsum
                            else:
                                qsum = workp.tile([128, 512], f16,
                                                  tag="qsum",
                                                  name="qsum")
                                nc.vector.tensor_add(
                                    qsum[:],
                                    den_state["heldsum"][:],
                                    ptsum[:])
                                den_state["heldsum"] = None
                                nc.tensor.matmul(
                                    dps[:], lhsT=onesm,
                                    rhs=qsum[:],
                                    start=not den_state["started"],
                                    stop=False)
                                den_state["started"] = True
                    elif (i - 4 * j) in (0, 2):
                        den_state["held"] = pt
                    else:
                        base = cols[i - 1]
                        dsum = workp.tile([128, 512], f16,
                                          tag="dsum", name="dsum")
                        nc.vector.tensor_add(
                            dsum[:, base:512],
                            den_state["held"][:, base:512],
                            pt[:, base:512])
                        den_state["held"] = None
                        nc.tensor.matmul(
                            dps[:, base:512], lhsT=onesm,
                            rhs=dsum[:, base:512],
                            start=not den_state["started"],
                            stop=last,
                            skip_group_check=(base > 0))
                        den_state["started"] = True
                    pts[i] = None

                off = min(4, nchunks - 1)
                for i in range(nchunks):
                    emit_score(i)
                    if i >= off:
                        emit_pv(i - off)
                    yield
                for i in range(nchunks - off, nchunks):
                    emit_pv(i)
                    yield
                rec32 = workp.tile([1, 512], f32, tag="rec32")
                nc.vector.reciprocal_approx_fast(rec32[:], dps[:])
                rec = workp.tile([1, 512], f16, tag="rec")
                nc.vector.tensor_copy(rec[:], rec32[:])

                def finish(h=h, j=j, zps=zps, rec=rec):
                    # broadcast of the reciprocal row across partitions on
                    # the (otherwise idle) GpSimd engine: frees the PE
                    # matmul + PSUM bank + DVE cast the old PE-broadcast
                    # needed
                    bsb = workp.tile([128, 512], f16, tag="bsb")
                    nc.gpsimd.partition_broadcast(bsb[:], rec[:])
                    z = postp.tile([128, 512], f16, tag=f"zn{h}",
                                   name=f"zn{h}_{j}")
                    nc.vector.tensor_mul(z[:], zps[:], bsb[:])
                    zn[h][j] = z

                defer.append([3, finish])
                yield

            # ------------- output-projection block generator -------------
            # 64 matmuls (16 groups of 4-head accumulation); yields after
            # each matmul. Copy+DMA per group emitted inline. The final
            # group of the final block splits its copy+DMA in two halves
            # issued on separate queues to shorten the exposed tail.
            def out_block(j, pool, fine_tail=False,
                          scalar_copies_from=99):
                Copy = mybir.ActivationFunctionType.Copy
                gidx = [0]

                def copy_out(osb_ap, src_ap):
                    # tail-region copies run on the (idle) ScalarE so the
                    # DVE FIFO holds only the recip->mul chain of the
                    # final head and never head-of-line blocks the PSUM
                    # bank recycling of these groups
                    if gidx[0] >= scalar_copies_from or fine_tail:
                        nc.scalar.activation(osb_ap, src_ap, Copy)
                    else:
                        nc.vector.tensor_copy(osb_ap, src_ap)

                def mm(ops, st, db, h):
                    nc.tensor.matmul(
                        ops[:],
                        lhsT=zn[h][j][:, st * 128:(st + 1) * 128],
                        rhs=woT_sb[:, h, db * 512:(db + 1) * 512],
                        start=(h == 0), stop=(h == HPC - 1))

                held = {}
                if fine_tail:
                    # fill both PSUM banks with the h0-2 partials of the
                    # first two groups while the last head's zn normalize
                    # chain (recip->broadcast->mul) completes
                    for db in range(2):
                        held[db] = pool.tile([128, 512], f32, tag="o",
                                             name="ops")
                        for h in range(HPC - 1):
                            mm(held[db], 0, db, h)
                            yield
                for st in range(4):
                    for db in range(4):
                        if (st, db) in ((0, 0), (0, 1)) and fine_tail:
                            ops = held[db]
                            mm(ops, st, db, HPC - 1)
                            yield
                        else:
                            ops = pool.tile([128, 512], f32, tag="o",
                                            name="ops")
                            for h in range(HPC):
                                mm(ops, st, db, h)
                                yield
                        row = j * 512 + st * 128
                        last = fine_tail and st == 3 and db == 3
                        gidx[0] += 1
                        if last:
                            for piece, eng in ((0, nc.sync),
                                               (1, nc.scalar)):
                                osb = osbp.tile([128, 256], f16,
                                                tag="osbf", name="osbf")
                                lo = piece * 256
                                copy_out(osb[:], ops[:, lo:lo + 256])
                                eng.dma_start(
                                    outp_d[row:row + 128,
                                           db * 512 + lo:
                                           db * 512 + lo + 256], osb[:])
                        else:
                            osb = osbp.tile([128, 512], f16, tag="osb",
                                            name="osb")
                            copy_out(osb[:], ops[:])
                            nc.sync.dma_start(
                                outp_d[row:row + 128,
                                       db * 512:(db + 1) * 512], osb[:])

            with (
                tc.tile_pool(name="psS", bufs=3, space="PSUM") as psS,
                tc.tile_pool(name="psZ", bufs=2, space="PSUM") as psZ,
                tc.tile_pool(name="psM", bufs=1, space="PSUM") as psM,
            ):
                # ---- Phase B: projections, with attn(0) interleaved
                with (
                    tc.tile_pool(name="big", bufs=1) as bigp,
                    tc.tile_pool(name="psumB", bufs=2,
                                 space="PSUM") as psB,
                ):
                    # xT is a per-j-block rotating buffer (bufs=2): block
                    # j's slice is only read by projection block j, and
                    # the j>=2 loads go on the SYNC queue only — they wait
                    # for proj(j-2) consumers, and putting them on the
                    # scalar queue would stall attn(0)'s exp stream
                    # behind them (deadlock against PE program order).
                    src = xT_d.rearrange("(c p) s -> p c s", p=128)
                    xTt = [bigp.tile([128, NC_CHUNKS, 512], f16, tag="xT",
                                     bufs=2, name=f"xT{j}")
                           for j in range(NBLK)]
                    w_sb = {}
                    for name, dram in (("wq", wq_d), ("wk", wk_d),
                                       ("wv", wv_d)):
                        w_sb[name] = bigp.tile([128, NC_CHUNKS, HE], f16,
                                               tag=name, name=name + "_sb")
                    wsrc = {name: dram.rearrange("(c p) n -> p c n", p=128)
                            for name, dram in
                            (("wq", wq_d), ("wk", wk_d), ("wv", wv_d))}

                    def load_w(eng, name, piece, npieces=2):
                        w = NC_CHUNKS // npieces
                        eng.dma_start(
                            w_sb[name][:, w * piece:w * piece + w, :],
                            wsrc[name][:, w * piece:w * piece + w, :])

                    def load_x(eng, j, half):
                        eng.dma_start(
                            xTt[j][:, 8 * half:8 * half + 8, :],
                            src[:, 8 * half:8 * half + 8,
                                j * 512:(j + 1) * 512])

                    # dummy matmuls on the ones tile while input DMAs run:
                    # keeps the PE-HAM activity window busy so the real
                    # matmul stream starts at the full 2.4GHz clock
                    for w in range(24):
                        wps = psB.tile([1, 128], f32, tag="proj",
                                       name="wps")
                        nc.tensor.matmul(wps[:], lhsT=onesm,
                                         rhs=ones_sb[:, 1:129],
                                         start=True, stop=True)

                    # first-needed data in small pieces, issued from BOTH
                    # hwdge queues (sync + scalar) so neither the ~700ns
                    # per-issue cost nor one queue's serial drain gates
                    # the first matmul chain
                    for c in range(4):
                        nc.sync.dma_start(w_sb["wk"][:, c:c + 1, :],
                                          wsrc["wk"][:, c:c + 1, :])
                        nc.scalar.dma_start(xTt[0][:, c:c + 1, :],
                                            src[:, c:c + 1, 0:512])
                    # second warm-up batch, gated on the first xT chunk so
                    # PE activity bridges the gap until projections begin
                    for w in range(16):
                        wps = psB.tile([1, 128], f32, tag="proj",
                                       name="wps")
                        nc.tensor.matmul(wps[:], lhsT=onesm,
                                         rhs=xTt[0][:, 0, 0:128],
                                         start=True, stop=True)
                    for e6 in range(2, 8):
                        nc.sync.dma_start(
                            w_sb["wk"][:, 2 * e6:2 * e6 + 2, :],
                            wsrc["wk"][:, 2 * e6:2 * e6 + 2, :])
                        nc.scalar.dma_start(
                            xTt[0][:, 2 * e6:2 * e6 + 2, :],
                            src[:, 2 * e6:2 * e6 + 2, 0:512])
                    for q in range(4):
                        load_w(nc.sync, "wq", q, 4)
                    nc.scalar.dma_start(masks_sb[:], masks_d[:])
                    for q in range(4):
                        load_w(nc.scalar, "wv", q, 4)
                    load_x(nc.sync, 1, 0)
                    load_x(nc.scalar, 1, 1)
                    wot_src = woT_d.rearrange("(c p) d -> p c d", p=128)
                    for c in range(HPC):
                        nc.sync.dma_start(woT_sb[:, c, :],
                                          wot_src[:, c, :])
                    for j in range(2, NBLK):
                        load_x(nc.sync, j, 0)
                        load_x(nc.sync, j, 1)

                    def proj(dst_ap, lhs_of_c, rhs_of_c):
                        ps = psB.tile([128, 512], f32, tag="proj",
                                      name="ps")
                        for c in range(NC_CHUNKS):
                            nc.tensor.matmul(
                                ps[:], lhsT=lhs_of_c(c), rhs=rhs_of_c(c),
                                start=(c == 0), stop=(c == NC_CHUNKS - 1))
                        nc.vector.tensor_copy(dst_ap, ps[:])

                    # attention block 0 interleaves into proj blocks 1..3:
                    # one attention quantum between projection chains
                    defer0 = []
                    gen0s = [attn_head(0, h, psS, psZ, psM, defer0)
                             for h in range(HPC)]
                    gen0i = 0

                    def step0():
                        nonlocal gen0i
                        while gen0i < len(gen0s):
                            if next(gen0s[gen0i], StopIteration) \
                                    is not StopIteration:
                                tick(defer0)
                                return
                            gen0i += 1

                    for j in range(NBLK):
                        if j == 0:
                            kq_order = [(kT[h], w_sb["wk"], h)
                                        for h in range(HPC)]
                            kq_order += [(qT[h], w_sb["wq"], h)
                                         for h in range(HPC)]
                        else:
                            kq_order = [p for h in range(HPC)
                                        for p in ((kT[h], w_sb["wk"], h),
                                                  (qT[h], w_sb["wq"], h))]
                        for dst, w, h in kq_order:
                            proj(dst[:, j * 512:(j + 1) * 512],
                                 lambda c, w=w, h=h:
                                 w[:, c, h * E:(h + 1) * E],
                                 lambda c, j=j: xTt[j][:, c, :])
                            if j >= 1:
                                step0()
                        for m in range(4 * j, 4 * j + 4):
                            proj(vt[m][:],
                                 lambda c, m=m, j=j:
                                 xTt[j][:, c,
                                        (m - 4 * j) * 128:
                                        (m - 4 * j + 1) * 128],
                                 lambda c: w_sb["wv"][:, c, :])
                            if j >= 1:
                                step0()
                    # drain whatever is left of attn(0)
                    while gen0i < len(gen0s):
                        step0()
                    flush(defer0)

                # ---- attention blocks 1..3 with out(j-1) interleaved,
                # then out(3) as a clean PE-bound tail
                with tc.tile_pool(name="psO", bufs=2,
                                  space="PSUM") as psO:
                    for j in range(1, NBLK):
                        og = out_block(j - 1, psO,
                                       scalar_copies_from=13
                                       if j == NBLK - 1 else 99)
                        odone = 0
                        # 72/64 pacing: out(j-1) finishes ~88% through the
                        # stage so its tail never collides with the next
                        # stage's start
                        nyield = 4 * (4 * j + 4 + off_yields(j))
                        ydone = 0
                        defer = []
                        for h in range(HPC):
                            for _ in attn_head(j, h, psS, psZ, psM,
                                               defer):
                                ydone += 1
                                tick(defer)
                                # stage 3 holds back ~8 out(2) matmuls:
                                # they are drained right after the last
                                # attention chunk, filling the PE while
                                # the final head's recip->broadcast->mul
                                # chain (~3us serial) completes
                                cap = 52 if j == NBLK - 1 else 64
                                target = min(cap, (72 * ydone) // nyield)
                                while odone < target:
                                    if next(og, StopIteration) \
                                            is StopIteration:
                                        odone = 64
                                    else:
                                        odone += 1
                        for _ in og:
                            pass
                        flush(defer)
                    for _ in out_block(NBLK - 1, psO, fine_tail=True):
                        pass

    nc.compile()
    return nc


def tick(defer):
    """Advance deferred emissions by one PE quantum; fire due ones."""
    for it in defer:
        it[0] -= 1
    while defer and defer[0][0] <= 0:
        defer.pop(0)[1]()


def flush(defer):
    while defer:
        defer.pop(0)[1]()


def off_yields(j):
    """Extra yields of attn_head(j, h) beyond one per chunk: the
    pipeline-drain PVs (off) plus the end-of-head chain yield."""
    return min(4, 4 * j + 3) + 1


def _get_nc():
    if "nc" not in _CACHE:
        _CACHE["nc"] = _build_program()
    return _CACHE["nc"]


def _host_inputs(x, W_Q, W_K, W_V, W_O):
    """Per-core input dicts (all fp16, pre-transposed)."""
    cc = np.arange(128)[None, :]
    mm = np.arange(128)[:, None]
    masks = (cc >= mm).astype(np.float16)

    in_maps = []
    for c in range(N_CORES):
        b, g = divmod(c, 4)
        hs = slice(HPC * g, HPC * g + HPC)
        xT = np.ascontiguousarray(x[b].T).astype(np.float16)
        wq = np.ascontiguousarray(
            W_Q[hs].transpose(2, 0, 1).reshape(D, HE)).astype(np.float16)
        wk = np.ascontiguousarray(
            W_K[hs].transpose(2, 0, 1).reshape(D, HE)).astype(np.float16)
        wv = np.ascontiguousarray(
            W_V[hs].transpose(2, 0, 1).reshape(D, HE)).astype(np.float16)
        woT = np.ascontiguousarray(
            W_O[hs].transpose(0, 2, 1).reshape(HE, D)).astype(np.float16)
        in_maps.append({"xT": xT, "wq": wq, "wk": wk, "wv": wv,
                        "woT": woT, "masks": masks})
    return in_maps


def _run(in_maps, trace=False, **kw):
    from concourse.bass_utils import run_bass_kernel_spmd
    nc = _get_nc()
    return run_bass_kernel_spmd(nc, in_maps, list(range(N_CORES)),
                                trace=trace, **kw)


def kernel(x, W_Q, W_K, W_V, W_O):
    x, W_Q, W_K, W_V, W_O = (np.asarray(a, dtype=np.float32)
                             for a in (x, W_Q, W_K, W_V, W_O))
    res = _run(_host_inputs(x, W_Q, W_K, W_V, W_O))
    parts = [np.asarray(res.results[c]["outp"], dtype=np.float32)
             for c in range(N_CORES)]
    out = np.stack([parts[0] + parts[1] + parts[2] + parts[3],
                    parts[4] + parts[5] + parts[6] + parts[7]])
    return out
